# revision 9
# baseline (speedup 1.0000x reference)
"""Causal linear multi-head attention (decoupled phi) on 8 trn2 NeuronCores.

Sharding: core c handles batch b = c//4 and head group hg = c%4 (4 of 16 heads).
Each core computes qkv projections for its heads, chunked causal linear
attention, and a partial output projection over its 256 feature columns.
Host sums the 4 partials per batch and adds out_b.

v2 layout/schedule notes (all bf16):
  - qT/kT: plain stacked (128 = 2 heads x 64 feats, L) tiles per head pair.
    A^T per head via 64-partition half matmuls (lhsT = kT half, rhs = qT half).
  - state S kept resident in PSUM (128, 260) accumulating across chunks
    (per pair: diag blocks [S|kcum] valid, off-diag garbage never read).
    Per chunk 4 small ACT copies build the block-diagonal bf16 Sblk used by
    the 2 pair inter matmuls (130 cols: [num|den] via [V|1] / [S|kcum]).
  - intra: 4 per-head matmuls (65 cols) vs masked A^T; den rides as 65th col.
  - scale split DVE/ACT; PE transpose to feature-major; out-proj interleaved
    stripe-wise with attention so the output DMA drains during compute.
  - x loaded in 512-l stripes so the first projection starts early.
    DMA queues: scalar = inputs + half outputs, sync = kl transposes +
    half outputs. kl (l-major k for the state matmul) via SBUF-SBUF DMA
    transpose.
"""

import numpy as np
import ml_dtypes

BF = ml_dtypes.bfloat16

B, L, E, H, D = 2, 2048, 1024, 16, 64
HC = 4            # heads per core
NCORES = 8
CH = 128          # chunk length
NCH = L // CH     # 16 chunks
LCH = 512         # l stripe for projections
NLC = L // LCH    # 4
ET = E // 128     # 8 e-tiles
DEPS = 1e-6

PROFILE = False
_STATE = {}


def _build():
    from contextlib import ExitStack
    from concourse import bacc, tile, mybir

    f32 = mybir.dt.float32
    bf16 = mybir.dt.bfloat16

    nc = bacc.Bacc("TRN2", target_bir_lowering=False, debug=False,
                   num_devices=NCORES)

    xT_d = nc.dram_tensor("xT", [E, L], bf16, kind="ExternalInput").ap()
    wqk_d = nc.dram_tensor("wqk", [E, 512], bf16, kind="ExternalInput").ap()
    bqk_d = nc.dram_tensor("bqk", [128, 4], f32, kind="ExternalInput").ap()
    wv_d = nc.dram_tensor("wv", [E, 256], bf16, kind="ExternalInput").ap()
    bv_d = nc.dram_tensor("bv", [128, 256], bf16, kind="ExternalInput").ap()
    wo_d = nc.dram_tensor("wo", [256, E], bf16, kind="ExternalInput").ap()
    mask_d = nc.dram_tensor("mask", [128, 512], f32, kind="ExternalInput").ap()
    ident_d = nc.dram_tensor("ident", [128, 128], bf16, kind="ExternalInput").ap()
    outT_d = nc.dram_tensor("outT", [E, L], bf16, kind="ExternalOutput").ap()

    with tile.TileContext(nc) as tc, ExitStack() as ctx:
        persist = ctx.enter_context(tc.tile_pool(name="persist", bufs=1))
        ps_big = ctx.enter_context(tc.tile_pool(name="psbig", bufs=2, space="PSUM"))
        ps_med = ctx.enter_context(tc.tile_pool(name="psmed", bufs=2, space="PSUM"))
        ps_n = ctx.enter_context(tc.tile_pool(name="psn", bufs=2, space="PSUM"))
        ps_st = ctx.enter_context(tc.tile_pool(name="psst", bufs=1, space="PSUM"))
        work = ctx.enter_context(tc.tile_pool(name="work", bufs=3))
        workb = ctx.enter_context(tc.tile_pool(name="workb", bufs=6))

        def pt(shape, dt, tag):
            return persist.tile(shape, dt, tag=tag, name=tag)

        # ---- persistent tiles ----
        xT = [pt([128, L], bf16, f"xT{i}") for i in range(ET)]
        wqk = [pt([128, 512], bf16, f"wqk{i}") for i in range(ET)]
        wv = [pt([128, 256], bf16, f"wv{i}") for i in range(ET)]
        wo = [pt([128, E], bf16, f"wo{i}") for i in range(2)]
        bqk = pt([128, 4], f32, "bqk")
        bv = pt([128, 256], bf16, "bv")
        mask = pt([128, 512], f32, "mask")
        ident = pt([128, 128], bf16, "ident")

        qT = [pt([128, L], bf16, f"qT{i}") for i in range(2)]
        kT = [pt([128, L], bf16, f"kT{i}") for i in range(2)]
        # odd-head halves shifted to base partition 0 (matmul operands at
        # base partition 64 are not safe on hw)
        qTo = [pt([64, L], bf16, f"qTo{i}") for i in range(2)]
        kTo = [pt([64, L], bf16, f"kTo{i}") for i in range(2)]
        kl = [pt([128, 256], bf16, f"kl{i}") for i in range(NCH - 1)]
        vs = [pt([128, 260], bf16, f"vs{i}") for i in range(NCH)]
        attnT = [pt([128, L], bf16, f"attnT{i}") for i in range(2)]
        Sblk = [pt([128, 260], bf16, f"Sblk{i}") for i in range(2)]

        # ---- input DMAs ----
        # scalar queue: inputs, ordered so the first v/qk stripes can start
        # early. sync queue: reserved for kl transposes (plus half the output
        # DMAs at the end).
        for et in range(ET):
            q = nc.sync if et % 2 == 0 else nc.scalar
            q.dma_start(xT[et][:, 0:LCH], xT_d[128 * et:128 * (et + 1), 0:LCH])
        for et in range(ET):
            q = nc.sync if et % 2 == 0 else nc.scalar
            q.dma_start(wqk[et][:], wqk_d[128 * et:128 * (et + 1), :])
        for et in range(ET):
            q = nc.sync if et % 2 == 0 else nc.scalar
            q.dma_start(wv[et][:], wv_d[128 * et:128 * (et + 1), :])
        nc.scalar.dma_start(bqk[:], bqk_d[:])
        nc.scalar.dma_start(bv[:], bv_d[:])
        nc.scalar.dma_start(mask[:], mask_d[:])
        nc.scalar.dma_start(ident[:], ident_d[:])
        for lc in range(1, NLC):
            lsl = slice(LCH * lc, LCH * (lc + 1))
            for et in range(ET):
                nc.scalar.dma_start(xT[et][:, lsl],
                                    xT_d[128 * et:128 * (et + 1), lsl])
        for i in range(2):
            nc.scalar.dma_start(wo[i][:], wo_d[128 * i:128 * (i + 1), :])

        # zero-init: Sblk off-diagonal stays zero forever; vs ones columns
        for i in range(2):
            nc.gpsimd.memset(Sblk[i][:], 0.0)
        for lt in range(NCH):
            v3 = vs[lt].rearrange("p (h w) -> p h w", w=65)
            nc.gpsimd.memset(v3[:, :, 64:65], 1.0)

        # state psum: pair t diag blocks [S|kcum] at cols 130t (+65 for odd)
        psS = ps_st.tile([128, 260], f32, tag="S")

        # ---- emission helpers ----
        def v_tile(lt):
            ps = ps_med.tile([128, 256], f32, tag="med")
            for et in range(ET):
                nc.tensor.matmul(ps[:], xT[et][:, 128 * lt:128 * (lt + 1)],
                                 wv[et][:], start=(et == 0), stop=(et == ET - 1))
            v3 = vs[lt].rearrange("p (h w) -> p h w", w=65)
            nc.vector.tensor_add(v3[:, :, 0:64],
                                 ps.rearrange("p (h w) -> p h w", w=64),
                                 bv.rearrange("p (h w) -> p h w", w=64))

        def qk_stripe(lc):
            lsl = slice(LCH * lc, LCH * (lc + 1))
            for ct in range(4):
                ps = ps_big.tile([128, LCH], f32, tag="big")
                for et in range(ET):
                    nc.tensor.matmul(
                        ps[:], wqk[et][:, 128 * ct:128 * (ct + 1)],
                        xT[et][:, lsl], start=(et == 0), stop=(et == ET - 1))
                dst = qT[ct] if ct < 2 else kT[ct - 2]
                nc.scalar.activation(dst[:, lsl], ps[:],
                                     mybir.ActivationFunctionType.Relu,
                                     bias=bqk[:, ct:ct + 1])
                dsto = qTo[ct] if ct < 2 else kTo[ct - 2]
                nc.sync.dma_start(dsto[:, lsl], dst[64:128, lsl])

        def kl_transposes(lc):
            for lt in range(4 * lc, min(4 * (lc + 1), NCH - 1)):
                for t in range(2):
                    nc.sync.dma_start_transpose(
                        kl[lt][:, 128 * t:128 * (t + 1)],
                        kT[t][:, 128 * lt:128 * (lt + 1)])

        def att_chunk(c):
            csl = slice(128 * c, 128 * (c + 1))
            # A^T per head: (128 lk, 128 lq), 64-feat contraction halves
            psA = ps_big.tile([128, 512], f32, tag="big")
            for h in range(HC):
                t, s = h // 2, h % 2
                if s == 0:
                    lh, rh = kT[t][0:64, csl], qT[t][0:64, csl]
                else:
                    lh, rh = kTo[t][:, csl], qTo[t][:, csl]
                nc.tensor.matmul(psA[:, 128 * h:128 * (h + 1)], lh, rh,
                                 start=(h == 0), stop=(h == HC - 1))
            AmT = work.tile([128, 512], bf16, tag="AmT")
            nc.vector.tensor_mul(AmT[:], psA[:], mask[:])
            # block-diag bf16 state [S|kcum] for this chunk's inter matmuls
            if c > 0:
                Sb = Sblk[c % 2]
                for h in range(HC):
                    t, s = h // 2, h % 2
                    rsl = slice(64 * s, 64 * (s + 1))
                    csl2 = slice(130 * t + 65 * s, 130 * t + 65 * (s + 1))
                    nc.scalar.copy(Sb[rsl, csl2], psS[rsl, csl2])
            # num/den: intra per head then inter per pair into one psum
            psn = ps_n.tile([128, 260], f32, tag="n")
            for h in range(HC):
                nc.tensor.matmul(psn[:, 65 * h:65 * (h + 1)],
                                 AmT[:, 128 * h:128 * (h + 1)],
                                 vs[c][:, 65 * h:65 * (h + 1)],
                                 start=(h == 0),
                                 stop=(c == 0 and h == HC - 1))
            if c > 0:
                for t in range(2):
                    nc.tensor.matmul(psn[:, 130 * t:130 * (t + 1)],
                                     qT[t][:, csl],
                                     Sblk[c % 2][:, 130 * t:130 * (t + 1)],
                                     start=False, stop=(t == 1))
            # state update for chunk c: the psum group must close every chunk
            # (psum cannot be read mid-group), so re-inject the previous
            # state via an identity matmul, then add chunk c's outer products.
            if c < NCH - 1:
                if c > 0:
                    for t in range(2):
                        nc.tensor.matmul(psS[:, 130 * t:130 * (t + 1)],
                                         ident[:],
                                         Sblk[c % 2][:, 130 * t:130 * (t + 1)],
                                         start=(t == 0), stop=False)
                for t in range(2):
                    nc.tensor.matmul(psS[:, 130 * t:130 * (t + 1)],
                                     kl[c][:, 128 * t:128 * (t + 1)],
                                     vs[c][:, 130 * t:130 * (t + 1)],
                                     start=(c == 0 and t == 0),
                                     stop=(t == 1))
            # dens -> reciprocal -> scale (split DVE/ACT)
            d4 = work.tile([128, 4], f32, tag="d4")
            nc.vector.tensor_scalar_max(
                d4[:],
                psn.rearrange("p (h w) -> p h w", w=65)[:, :, 64:65].opt(),
                DEPS)
            r4 = work.tile([128, 4], f32, tag="r4")
            nc.vector.reciprocal(r4[:], d4[:])
            att = work.tile([128, 256], bf16, tag="att")
            for h in range(HC):
                dst = att[:, 64 * h:64 * (h + 1)]
                src = psn[:, 65 * h:65 * h + 64]
                sc = r4[:, h:h + 1]
                if h < 2:
                    nc.vector.tensor_scalar_mul(dst, src, sc)
                else:
                    nc.scalar.mul(dst, src, sc)
            # transpose to feature-major (separate psum tiles per half so the
            # copy of one half never races the other's accumulation group)
            psT0 = ps_med.tile([128, 128], bf16, tag="medT", bufs=1)
            nc.tensor.transpose(psT0[:], att[:, 0:128], ident[:])
            nc.scalar.copy(attnT[0][:, csl], psT0[:])
            psT1 = ps_med.tile([128, 128], bf16, tag="medT", bufs=1)
            nc.tensor.transpose(psT1[:], att[:, 128:256], ident[:])
            nc.vector.tensor_copy(attnT[1][:, csl], psT1[:])

        def out_stripe(lc):
            lsl = slice(LCH * lc, LCH * (lc + 1))
            for ot in range(ET):
                ps = ps_big.tile([128, LCH], f32, tag="big")
                for eb in range(2):
                    nc.tensor.matmul(ps[:], wo[eb][:, 128 * ot:128 * (ot + 1)],
                                     attnT[eb][:, lsl],
                                     start=(eb == 0), stop=(eb == 1))
                ob = workb.tile([128, LCH], bf16, tag="ob")
                if ot % 2 == 0:
                    nc.vector.tensor_copy(ob[:], ps[:])
                else:
                    nc.scalar.copy(ob[:], ps[:])
                q = nc.scalar if ot % 2 == 0 else nc.sync
                q.dma_start(outT_d[128 * ot:128 * (ot + 1), lsl], ob[:])

        # ---- emission order (the tile scheduler refines per-engine order) ----
        for lt in range(4):
            v_tile(lt)
        qk_stripe(0)
        kl_transposes(0)
        qk_stripe(1)
        for lt in range(4, 8):
            v_tile(lt)
        kl_transposes(1)
        for c in range(0, 4):
            att_chunk(c)
        qk_stripe(2)
        for lt in range(8, 12):
            v_tile(lt)
        kl_transposes(2)
        out_stripe(0)
        for c in range(4, 8):
            att_chunk(c)
        qk_stripe(3)
        for lt in range(12, 16):
            v_tile(lt)
        kl_transposes(3)
        out_stripe(1)
        for c in range(8, 12):
            att_chunk(c)
        out_stripe(2)
        for c in range(12, 16):
            att_chunk(c)
        out_stripe(3)

    nc.compile()
    return nc


def _prep_inputs(x, qkv_w, qkv_b, out_w):
    mask = np.tile(np.triu(np.ones((128, 128), np.float32)), (1, 4))
    ident = np.eye(128, dtype=np.float32).astype(BF)
    in_maps = []
    for c in range(NCORES):
        b, hg = c // 4, c % 4
        rows = np.arange(256 * hg, 256 * (hg + 1))
        wqk = np.concatenate([qkv_w[rows], qkv_w[rows + E]], 0).T
        bqk = np.concatenate([qkv_b[rows], qkv_b[rows + E]]).reshape(4, 128).T
        wv = qkv_w[rows + 2 * E].T
        bv = np.tile(qkv_b[rows + 2 * E][None, :], (128, 1))
        wo = out_w[:, rows].T
        in_maps.append({
            "xT": np.ascontiguousarray(x[b].T).astype(BF),
            "wqk": np.ascontiguousarray(wqk).astype(BF),
            "bqk": np.ascontiguousarray(bqk).astype(np.float32),
            "wv": np.ascontiguousarray(wv).astype(BF),
            "bv": np.ascontiguousarray(bv).astype(BF),
            "wo": np.ascontiguousarray(wo).astype(BF),
            "mask": mask, "ident": ident,
        })
    return in_maps


def kernel(x, qkv_w, qkv_b, out_w, out_b):
    from concourse.bass_utils import run_bass_kernel_spmd

    x = np.asarray(x, np.float32)
    qkv_w = np.asarray(qkv_w, np.float32)
    qkv_b = np.asarray(qkv_b, np.float32)
    out_w = np.asarray(out_w, np.float32)
    out_b = np.asarray(out_b, np.float32)

    if "nc" not in _STATE:
        _STATE["nc"] = _build()
    nc = _STATE["nc"]
    in_maps = _prep_inputs(x, qkv_w, qkv_b, out_w)
    res = run_bass_kernel_spmd(nc, in_maps, list(range(NCORES)),
                               trace=PROFILE)
    _STATE["last"] = res
    out = np.zeros((B, L, E), np.float32)
    for c in range(NCORES):
        out[c // 4] += res.results[c]["outT"].T
    out += out_b
    return out


# revision 19
# speedup vs baseline: 1.0593x; 1.0593x over previous
"""Causal linear multi-head attention (decoupled phi) on 8 trn2 NeuronCores.

Sharding: core c handles batch b = c//4 and head group hg = c%4 (4 of 16 heads).
Each core computes qkv projections for its heads, chunked causal linear
attention, and a partial output projection over its 256 feature columns.
Host sums the 4 partials per batch and adds out_b.

v2 layout/schedule notes (all bf16):
  - qT/kT: plain stacked (128 = 2 heads x 64 feats, L) tiles per head pair.
    A^T per head via 64-partition half matmuls (lhsT = kT half, rhs = qT half).
  - state S kept resident in PSUM (128, 260) accumulating across chunks
    (per pair: diag blocks [S|kcum] valid, off-diag garbage never read).
    Per chunk 4 small ACT copies build the block-diagonal bf16 Sblk used by
    the 2 pair inter matmuls (130 cols: [num|den] via [V|1] / [S|kcum]).
  - intra: 4 per-head matmuls (65 cols) vs masked A^T; den rides as 65th col.
  - scale split DVE/ACT; PE transpose to feature-major; out-proj interleaved
    stripe-wise with attention so the output DMA drains during compute.
  - x loaded in 512-l stripes so the first projection starts early.
    DMA queues: scalar = inputs + half outputs, sync = kl transposes +
    half outputs. kl (l-major k for the state matmul) via SBUF-SBUF DMA
    transpose.
"""

import numpy as np
import ml_dtypes

BF = ml_dtypes.bfloat16

B, L, E, H, D = 2, 2048, 1024, 16, 64
HC = 4            # heads per core
NCORES = 8
CH = 128          # chunk length
NCH = L // CH     # 16 chunks
LCH = 512         # l stripe for projections
NLC = L // LCH    # 4
ET = E // 128     # 8 e-tiles
DEPS = 1e-6

PROFILE = False
_STATE = {}


def _build():
    from contextlib import ExitStack
    from concourse import bacc, tile, mybir

    f32 = mybir.dt.float32
    bf16 = mybir.dt.bfloat16

    nc = bacc.Bacc("TRN2", target_bir_lowering=False, debug=False,
                   num_devices=NCORES)

    # weights host-packed et-major so each loads in ONE dma (fewer dma
    # semaphores; first matmul needs every e-tile anyway)
    xT_d = nc.dram_tensor("xT", [E, L], bf16, kind="ExternalInput").ap()
    wqk_d = nc.dram_tensor("wqk", [128, ET * 512], bf16, kind="ExternalInput").ap()
    bqk_d = nc.dram_tensor("bqk", [128, 4], f32, kind="ExternalInput").ap()
    wv_d = nc.dram_tensor("wv", [128, ET * 256], bf16, kind="ExternalInput").ap()
    bv_d = nc.dram_tensor("bv", [128, 256], bf16, kind="ExternalInput").ap()
    wo_d = nc.dram_tensor("wo", [128, 2 * E], bf16, kind="ExternalInput").ap()
    mask_d = nc.dram_tensor("mask", [128, 512], f32, kind="ExternalInput").ap()
    ident_d = nc.dram_tensor("ident", [128, 128], bf16, kind="ExternalInput").ap()
    outT_d = nc.dram_tensor("outT", [E, L], bf16, kind="ExternalOutput").ap()

    with tile.TileContext(nc) as tc, ExitStack() as ctx:
        persist = ctx.enter_context(tc.tile_pool(name="persist", bufs=1))
        ps_big = ctx.enter_context(tc.tile_pool(name="psbig", bufs=2, space="PSUM"))
        ps_med = ctx.enter_context(tc.tile_pool(name="psmed", bufs=2, space="PSUM"))
        ps_n = ctx.enter_context(tc.tile_pool(name="psn", bufs=2, space="PSUM"))
        ps_st = ctx.enter_context(tc.tile_pool(name="psst", bufs=1, space="PSUM"))
        work = ctx.enter_context(tc.tile_pool(name="work", bufs=3))
        workb = ctx.enter_context(tc.tile_pool(name="workb", bufs=6))

        def pt(shape, dt, tag):
            return persist.tile(shape, dt, tag=tag, name=tag)

        # ---- persistent tiles (et-major packed) ----
        xTt = pt([128, ET * L], bf16, "xTt")
        xv = xTt.rearrange("p (a l) -> p a l", l=L)
        wqkt = pt([128, ET * 512], bf16, "wqkt")
        wqkv = wqkt.rearrange("p (a c) -> p a c", c=512)
        wvt = pt([128, ET * 256], bf16, "wvt")
        wvv = wvt.rearrange("p (a c) -> p a c", c=256)
        wot = pt([128, 2 * E], bf16, "wot")
        wov = wot.rearrange("p (a c) -> p a c", c=E)
        bqk = pt([128, 4], f32, "bqk")
        bv = pt([128, 256], bf16, "bv")
        mask = pt([128, 512], f32, "mask")
        ident = pt([128, 128], bf16, "ident")

        qT = [pt([128, L], bf16, f"qT{i}") for i in range(2)]
        kT = [pt([128, L], bf16, f"kT{i}") for i in range(2)]
        # odd-head halves shifted to base partition 0 (matmul operands at
        # base partition 64 are not safe on hw)
        qTo = [pt([64, L], bf16, f"qTo{i}") for i in range(2)]
        kTo = [pt([64, L], bf16, f"kTo{i}") for i in range(2)]
        kl = [pt([128, 256], bf16, f"kl{i}") for i in range(NCH - 1)]
        vs = [pt([128, 260], bf16, f"vs{i}") for i in range(NCH)]
        attnT = [pt([128, L], bf16, f"attnT{i}") for i in range(2)]
        Sblk = [pt([128, 260], bf16, f"Sblk{i}") for i in range(2)]

        # ---- input DMAs (11 total, split across sync/scalar) ----
        # sync carries the kl transposes from ~7us on; scalar carries the
        # rest of the inputs then the early output stripes.
        xsrc = xT_d.rearrange("(a p) l -> p a l", p=128)
        nc.sync.dma_start(xv[:, :, 0:LCH], xsrc[:, :, 0:LCH])
        nc.scalar.dma_start(wqkt[:], wqk_d[:])
        nc.scalar.dma_start(bqk[:], bqk_d[:])
        nc.scalar.dma_start(bv[:], bv_d[:])
        nc.sync.dma_start(wvt[:], wv_d[:])
        nc.scalar.dma_start(mask[:], mask_d[:])
        nc.scalar.dma_start(ident[:], ident_d[:])
        nc.scalar.dma_start(xv[:, :, LCH:2 * LCH], xsrc[:, :, LCH:2 * LCH])
        nc.sync.dma_start(xv[:, :, 2 * LCH:3 * LCH], xsrc[:, :, 2 * LCH:3 * LCH])
        nc.scalar.dma_start(xv[:, :, 3 * LCH:4 * LCH], xsrc[:, :, 3 * LCH:4 * LCH])
        nc.scalar.dma_start(wot[:], wo_d[:])

        # zero-init: Sblk off-diagonal stays zero forever; vs ones columns
        for i in range(2):
            nc.gpsimd.memset(Sblk[i][:], 0.0)
        for lt in range(NCH):
            v3 = vs[lt].rearrange("p (h w) -> p h w", w=65)
            nc.gpsimd.memset(v3[:, :, 64:65], 1.0)

        # state psum: pair t diag blocks [S|kcum] at cols 130t (+65 for odd)
        psS = ps_st.tile([128, 260], f32, tag="S")

        # ---- emission helpers ----
        def v_tile(lt):
            ps = ps_med.tile([128, 256], f32, tag="med")
            for et in range(ET):
                nc.tensor.matmul(ps[:], xv[:, et, 128 * lt:128 * (lt + 1)],
                                 wvv[:, et, :], start=(et == 0), stop=(et == ET - 1))
            v3 = vs[lt].rearrange("p (h w) -> p h w", w=65)
            nc.vector.tensor_add(v3[:, :, 0:64],
                                 ps.rearrange("p (h w) -> p h w", w=64),
                                 bv.rearrange("p (h w) -> p h w", w=64))

        def qk_stripe(lc):
            lsl = slice(LCH * lc, LCH * (lc + 1))
            for ct in range(4):
                ps = ps_big.tile([128, LCH], f32, tag="big")
                for et in range(ET):
                    nc.tensor.matmul(
                        ps[:], wqkv[:, et, 128 * ct:128 * (ct + 1)],
                        xv[:, et, lsl], start=(et == 0), stop=(et == ET - 1))
                dst = qT[ct] if ct < 2 else kT[ct - 2]
                nc.scalar.activation(dst[:, lsl], ps[:],
                                     mybir.ActivationFunctionType.Relu,
                                     bias=bqk[:, ct:ct + 1])
                dsto = qTo[ct] if ct < 2 else kTo[ct - 2]
                nc.gpsimd.dma_start(dsto[:, lsl], dst[64:128, lsl])

        def kl_transposes(lc):
            for lt in range(4 * lc, min(4 * (lc + 1), NCH - 1)):
                for t in range(2):
                    nc.sync.dma_start_transpose(
                        kl[lt][:, 128 * t:128 * (t + 1)],
                        kT[t][:, 128 * lt:128 * (lt + 1)])

        def att_chunk(c):
            csl = slice(128 * c, 128 * (c + 1))
            # A^T per head: (128 lk, 128 lq), 64-feat contraction halves
            psA = ps_big.tile([128, 512], f32, tag="big")
            for h in range(HC):
                t, s = h // 2, h % 2
                if s == 0:
                    lh, rh = kT[t][0:64, csl], qT[t][0:64, csl]
                else:
                    lh, rh = kTo[t][:, csl], qTo[t][:, csl]
                nc.tensor.matmul(psA[:, 128 * h:128 * (h + 1)], lh, rh,
                                 start=(h == 0), stop=(h == HC - 1))
            AmT = work.tile([128, 512], bf16, tag="AmT")
            nc.vector.tensor_mul(AmT[:], psA[:], mask[:])
            # block-diag bf16 state [S|kcum] for this chunk's inter matmuls
            if c > 0:
                Sb = Sblk[c % 2]
                for s in range(2):
                    rsl = slice(64 * s, 64 * (s + 1))
                    v3d = lambda ap: ap[rsl, :].rearrange(
                        "p (t w) -> p t w", w=130)[:, :, 65 * s:65 * (s + 1)]
                    nc.scalar.copy(v3d(Sb), v3d(psS))
            # num/den: intra per head then inter per pair into one psum
            psn = ps_n.tile([128, 260], f32, tag="n")
            for h in range(HC):
                nc.tensor.matmul(psn[:, 65 * h:65 * (h + 1)],
                                 AmT[:, 128 * h:128 * (h + 1)],
                                 vs[c][:, 65 * h:65 * (h + 1)],
                                 start=(h == 0),
                                 stop=(c == 0 and h == HC - 1))
            if c > 0:
                for t in range(2):
                    nc.tensor.matmul(psn[:, 130 * t:130 * (t + 1)],
                                     qT[t][:, csl],
                                     Sblk[c % 2][:, 130 * t:130 * (t + 1)],
                                     start=False, stop=(t == 1))
            # state update for chunk c: the psum group must close every chunk
            # (psum cannot be read mid-group), so re-inject the previous
            # state via an identity matmul, then add chunk c's outer products.
            if c < NCH - 1:
                if c > 0:
                    for t in range(2):
                        nc.tensor.matmul(psS[:, 130 * t:130 * (t + 1)],
                                         ident[:],
                                         Sblk[c % 2][:, 130 * t:130 * (t + 1)],
                                         start=(t == 0), stop=False)
                for t in range(2):
                    nc.tensor.matmul(psS[:, 130 * t:130 * (t + 1)],
                                     kl[c][:, 128 * t:128 * (t + 1)],
                                     vs[c][:, 130 * t:130 * (t + 1)],
                                     start=(c == 0 and t == 0),
                                     stop=(t == 1))
            # dens -> reciprocal -> scale (split DVE/ACT)
            d4 = work.tile([128, 4], f32, tag="d4")
            nc.vector.tensor_scalar_max(
                d4[:],
                psn.rearrange("p (h w) -> p h w", w=65)[:, :, 64:65].opt(),
                DEPS)
            r4 = work.tile([128, 4], f32, tag="r4")
            nc.vector.reciprocal(r4[:], d4[:])
            att = work.tile([128, 256], bf16, tag="att")
            for h in range(HC):
                dst = att[:, 64 * h:64 * (h + 1)]
                src = psn[:, 65 * h:65 * h + 64]
                sc = r4[:, h:h + 1]
                if h < 2:
                    nc.vector.tensor_scalar_mul(dst, src, sc)
                else:
                    nc.scalar.mul(dst, src, sc)
            # transpose to feature-major (separate psum tiles per half so the
            # copy of one half never races the other's accumulation group)
            psT0 = ps_med.tile([128, 128], bf16, tag="medT", bufs=1)
            nc.tensor.transpose(psT0[:], att[:, 0:128], ident[:])
            nc.scalar.copy(attnT[0][:, csl], psT0[:])
            psT1 = ps_med.tile([128, 128], bf16, tag="medT", bufs=1)
            nc.tensor.transpose(psT1[:], att[:, 128:256], ident[:])
            nc.vector.tensor_copy(attnT[1][:, csl], psT1[:])

        def out_stripe(lc):
            lsl = slice(LCH * lc, LCH * (lc + 1))
            for ot in range(ET):
                ps = ps_big.tile([128, LCH], f32, tag="big")
                for eb in range(2):
                    nc.tensor.matmul(ps[:], wov[:, eb, 128 * ot:128 * (ot + 1)],
                                     attnT[eb][:, lsl],
                                     start=(eb == 0), stop=(eb == 1))
                ob = workb.tile([128, LCH], bf16, tag="ob")
                nc.vector.tensor_copy(ob[:], ps[:])
                q = nc.scalar if lc < 2 else nc.sync
                q.dma_start(outT_d[128 * ot:128 * (ot + 1), lsl], ob[:])

        # ---- emission order (the tile scheduler refines per-engine order) ----
        for lt in range(4):
            v_tile(lt)
        qk_stripe(0)
        kl_transposes(0)
        qk_stripe(1)
        for lt in range(4, 8):
            v_tile(lt)
        kl_transposes(1)
        for c in range(0, 4):
            att_chunk(c)
        qk_stripe(2)
        for lt in range(8, 12):
            v_tile(lt)
        kl_transposes(2)
        out_stripe(0)
        for c in range(4, 8):
            att_chunk(c)
        qk_stripe(3)
        for lt in range(12, 16):
            v_tile(lt)
        kl_transposes(3)
        out_stripe(1)
        for c in range(8, 12):
            att_chunk(c)
        out_stripe(2)
        for c in range(12, 16):
            att_chunk(c)
        out_stripe(3)

    nc.compile()
    return nc


def _prep_inputs(x, qkv_w, qkv_b, out_w):
    mask = np.tile(np.triu(np.ones((128, 128), np.float32)), (1, 4))
    ident = np.eye(128, dtype=np.float32).astype(BF)
    in_maps = []
    def etpack(w):
        # (E, C) -> (128, ET*C): et-major packing, [p, et*C + c] = w[128et+p, c]
        e, cc = w.shape
        return w.reshape(e // 128, 128, cc).transpose(1, 0, 2).reshape(128, -1)

    for c in range(NCORES):
        b, hg = c // 4, c % 4
        rows = np.arange(256 * hg, 256 * (hg + 1))
        wqk = np.concatenate([qkv_w[rows], qkv_w[rows + E]], 0).T
        bqk = np.concatenate([qkv_b[rows], qkv_b[rows + E]]).reshape(4, 128).T
        wv = qkv_w[rows + 2 * E].T
        bv = np.tile(qkv_b[rows + 2 * E][None, :], (128, 1))
        wo = out_w[:, rows].T
        in_maps.append({
            "xT": np.ascontiguousarray(x[b].T).astype(BF),
            "wqk": np.ascontiguousarray(etpack(wqk)).astype(BF),
            "bqk": np.ascontiguousarray(bqk).astype(np.float32),
            "wv": np.ascontiguousarray(etpack(wv)).astype(BF),
            "bv": np.ascontiguousarray(bv).astype(BF),
            "wo": np.ascontiguousarray(etpack(wo)).astype(BF),
            "mask": mask, "ident": ident,
        })
    return in_maps


def kernel(x, qkv_w, qkv_b, out_w, out_b):
    from concourse.bass_utils import run_bass_kernel_spmd

    x = np.asarray(x, np.float32)
    qkv_w = np.asarray(qkv_w, np.float32)
    qkv_b = np.asarray(qkv_b, np.float32)
    out_w = np.asarray(out_w, np.float32)
    out_b = np.asarray(out_b, np.float32)

    if "nc" not in _STATE:
        _STATE["nc"] = _build()
    nc = _STATE["nc"]
    in_maps = _prep_inputs(x, qkv_w, qkv_b, out_w)
    res = run_bass_kernel_spmd(nc, in_maps, list(range(NCORES)),
                               trace=PROFILE)
    _STATE["last"] = res
    out = np.zeros((B, L, E), np.float32)
    for c in range(NCORES):
        out[c // 4] += res.results[c]["outT"].T
    out += out_b
    return out


# revision 23
# speedup vs baseline: 1.1831x; 1.1169x over previous
"""Causal linear multi-head attention (decoupled phi) on 8 trn2 NeuronCores.

Sharding: core c handles batch b = c//4 and head group hg = c%4 (4 of 16 heads).
Each core computes qkv projections for its heads, chunked causal linear
attention, and a partial output projection over its 256 feature columns.
Host sums the 4 partials per batch and adds out_b.

v2 layout/schedule notes (all bf16):
  - qT/kT: plain stacked (128 = 2 heads x 64 feats, L) tiles per head pair.
    A^T per head via 64-partition half matmuls (lhsT = kT half, rhs = qT half).
  - state S kept resident in PSUM (128, 260) accumulating across chunks
    (per pair: diag blocks [S|kcum] valid, off-diag garbage never read).
    Per chunk 4 small ACT copies build the block-diagonal bf16 Sblk used by
    the 2 pair inter matmuls (130 cols: [num|den] via [V|1] / [S|kcum]).
  - intra: 4 per-head matmuls (65 cols) vs masked A^T; den rides as 65th col.
  - scale split DVE/ACT; PE transpose to feature-major; out-proj interleaved
    stripe-wise with attention so the output DMA drains during compute.
  - x loaded in 512-l stripes so the first projection starts early.
    DMA queues: scalar = inputs + half outputs, sync = kl transposes +
    half outputs. kl (l-major k for the state matmul) via SBUF-SBUF DMA
    transpose.
"""

import numpy as np
import ml_dtypes

BF = ml_dtypes.bfloat16

B, L, E, H, D = 2, 2048, 1024, 16, 64
HC = 4            # heads per core
NCORES = 8
CH = 128          # chunk length
NCH = L // CH     # 16 chunks
LCH = 512         # l stripe for projections
NLC = L // LCH    # 4
ET = E // 128     # 8 e-tiles
DEPS = 1e-6

PROFILE = False
_STATE = {}


def _build():
    from contextlib import ExitStack
    from concourse import bacc, tile, mybir

    f32 = mybir.dt.float32
    bf16 = mybir.dt.bfloat16

    nc = bacc.Bacc("TRN2", target_bir_lowering=False, debug=False,
                   num_devices=NCORES)

    # weights host-packed et-major so each loads in ONE dma (fewer dma
    # semaphores; first matmul needs every e-tile anyway)
    xT_d = nc.dram_tensor("xT", [E, L], bf16, kind="ExternalInput").ap()
    wqk_d = nc.dram_tensor("wqk", [128, ET * 512], bf16, kind="ExternalInput").ap()
    bqk_d = nc.dram_tensor("bqk", [128, 4], f32, kind="ExternalInput").ap()
    wv_d = nc.dram_tensor("wv", [128, ET * 256], bf16, kind="ExternalInput").ap()
    bv_d = nc.dram_tensor("bv", [128, 256], bf16, kind="ExternalInput").ap()
    wo_d = nc.dram_tensor("wo", [128, 2 * E], bf16, kind="ExternalInput").ap()
    mask_d = nc.dram_tensor("mask", [128, 512], f32, kind="ExternalInput").ap()
    ident_d = nc.dram_tensor("ident", [128, 128], bf16, kind="ExternalInput").ap()
    outT_d = nc.dram_tensor("outT", [E, L], bf16, kind="ExternalOutput").ap()

    with tile.TileContext(nc) as tc, ExitStack() as ctx:
        persist = ctx.enter_context(tc.tile_pool(name="persist", bufs=1))
        ps_big = ctx.enter_context(tc.tile_pool(name="psbig", bufs=2, space="PSUM"))
        ps_med = ctx.enter_context(tc.tile_pool(name="psmed", bufs=2, space="PSUM"))
        ps_n = ctx.enter_context(tc.tile_pool(name="psn", bufs=2, space="PSUM"))
        ps_st = ctx.enter_context(tc.tile_pool(name="psst", bufs=1, space="PSUM"))
        work = ctx.enter_context(tc.tile_pool(name="work", bufs=3))
        workb = ctx.enter_context(tc.tile_pool(name="workb", bufs=6))

        def pt(shape, dt, tag):
            return persist.tile(shape, dt, tag=tag, name=tag)

        # ---- persistent tiles (et-major packed) ----
        xTt = pt([128, ET * L], bf16, "xTt")
        xv = xTt.rearrange("p (a l) -> p a l", l=L)
        wqkt = pt([128, ET * 512], bf16, "wqkt")
        wqkv = wqkt.rearrange("p (a c) -> p a c", c=512)
        wvt = pt([128, ET * 256], bf16, "wvt")
        wvv = wvt.rearrange("p (a c) -> p a c", c=256)
        wot = pt([128, 2 * E], bf16, "wot")
        wov = wot.rearrange("p (a c) -> p a c", c=E)
        bqk = pt([128, 4], f32, "bqk")
        bv = pt([128, 256], bf16, "bv")
        mask = pt([128, 512], f32, "mask")
        ident = pt([128, 128], bf16, "ident")

        qT = [pt([128, L], bf16, f"qT{i}") for i in range(2)]
        kT = [pt([128, L], bf16, f"kT{i}") for i in range(2)]
        # odd-head halves shifted to base partition 0 (matmul operands at
        # base partition 64 are not safe on hw)
        qTo = [pt([64, L], bf16, f"qTo{i}") for i in range(2)]
        kTo = [pt([64, L], bf16, f"kTo{i}") for i in range(2)]
        # l-major k per stripe: klS[lc][p, j, t, w] = phi(k)[l=512lc+128j+p,
        # feat 128t+w]; filled by (batched) SBUF-SBUF dma transposes
        klS = [pt([128, 1024], bf16, f"klS{i}") for i in range(NLC)]
        klv = [klS[i].rearrange("p (j t w) -> p j t w", t=2, w=128)
               for i in range(NLC)]
        vs = [pt([128, 260], bf16, f"vs{i}") for i in range(NCH)]
        attnT = [pt([128, L], bf16, f"attnT{i}") for i in range(2)]
        Sblk = [pt([128, 260], bf16, f"Sblk{i}") for i in range(2)]

        # ---- input DMAs (11 total, split across sync/scalar) ----
        # sync carries the kl transposes from ~7us on; scalar carries the
        # rest of the inputs then the early output stripes.
        xsrc = xT_d.rearrange("(a p) l -> p a l", p=128)
        nc.sync.dma_start(xv[:, :, 0:LCH], xsrc[:, :, 0:LCH])
        nc.scalar.dma_start(wqkt[:], wqk_d[:])
        nc.scalar.dma_start(bqk[:], bqk_d[:])
        nc.scalar.dma_start(bv[:], bv_d[:])
        nc.sync.dma_start(wvt[:], wv_d[:])
        nc.scalar.dma_start(mask[:], mask_d[:])
        nc.scalar.dma_start(ident[:], ident_d[:])
        nc.scalar.dma_start(xv[:, :, LCH:2 * LCH], xsrc[:, :, LCH:2 * LCH])
        nc.sync.dma_start(xv[:, :, 2 * LCH:3 * LCH], xsrc[:, :, 2 * LCH:3 * LCH])
        nc.scalar.dma_start(xv[:, :, 3 * LCH:4 * LCH], xsrc[:, :, 3 * LCH:4 * LCH])
        nc.scalar.dma_start(wot[:], wo_d[:])

        # zero-init: Sblk off-diagonal stays zero forever; vs ones columns
        for i in range(2):
            nc.gpsimd.memset(Sblk[i][:], 0.0)
        for lt in range(NCH):
            v3 = vs[lt].rearrange("p (h w) -> p h w", w=65)
            nc.gpsimd.memset(v3[:, :, 64:65], 1.0)

        # state psum: pair t diag blocks [S|kcum] at cols 130t (+65 for odd)
        psS = ps_st.tile([128, 260], f32, tag="S")

        # ---- emission helpers ----
        def v_tile(lt):
            ps = ps_med.tile([128, 256], f32, tag="med")
            for et in range(ET):
                nc.tensor.matmul(ps[:], xv[:, et, 128 * lt:128 * (lt + 1)],
                                 wvv[:, et, :], start=(et == 0), stop=(et == ET - 1))
            v3 = vs[lt].rearrange("p (h w) -> p h w", w=65)
            nc.vector.tensor_add(v3[:, :, 0:64],
                                 ps.rearrange("p (h w) -> p h w", w=64),
                                 bv.rearrange("p (h w) -> p h w", w=64))

        def qk_stripe(lc):
            lsl = slice(LCH * lc, LCH * (lc + 1))
            for ct in range(4):
                ps = ps_big.tile([128, LCH], f32, tag="big")
                for et in range(ET):
                    nc.tensor.matmul(
                        ps[:], wqkv[:, et, 128 * ct:128 * (ct + 1)],
                        xv[:, et, lsl], start=(et == 0), stop=(et == ET - 1))
                dst = qT[ct] if ct < 2 else kT[ct - 2]
                nc.scalar.activation(dst[:, lsl], ps[:],
                                     mybir.ActivationFunctionType.Relu,
                                     bias=bqk[:, ct:ct + 1])
                dsto = qTo[ct] if ct < 2 else kTo[ct - 2]
                nc.gpsimd.dma_start(dsto[:, lsl], dst[64:128, lsl])

        def kl_transposes(lc):
            if lc == 0:
                # per-chunk for latency: chunk 0's state matmul is needed early
                for j in range(4):
                    for t in range(2):
                        nc.sync.dma_start_transpose(
                            klv[0][:, j, t, :],
                            kT[t][:, 128 * j:128 * (j + 1)])
            else:
                for t in range(2):
                    nc.sync.dma_start_transpose(
                        klv[lc][:, :, t, :],
                        kT[t][:, LCH * lc:LCH * (lc + 1)])

        def att_chunk(c):
            csl = slice(128 * c, 128 * (c + 1))
            # A^T per head: (128 lk, 128 lq), 64-feat contraction halves
            psA = ps_big.tile([128, 512], f32, tag="big")
            for h in range(HC):
                t, s = h // 2, h % 2
                if s == 0:
                    lh, rh = kT[t][0:64, csl], qT[t][0:64, csl]
                else:
                    lh, rh = kTo[t][:, csl], qTo[t][:, csl]
                nc.tensor.matmul(psA[:, 128 * h:128 * (h + 1)], lh, rh,
                                 start=(h == 0), stop=(h == HC - 1))
            AmT = work.tile([128, 512], bf16, tag="AmT")
            nc.vector.tensor_mul(AmT[:], psA[:], mask[:])
            # block-diag bf16 state [S|kcum] for this chunk's inter matmuls
            if c > 0:
                Sb = Sblk[c % 2]
                for s in range(2):
                    rsl = slice(64 * s, 64 * (s + 1))
                    v3d = lambda ap: ap[rsl, :].rearrange(
                        "p (t w) -> p t w", w=130)[:, :, 65 * s:65 * (s + 1)]
                    nc.scalar.copy(v3d(Sb), v3d(psS))
            # num/den: intra per head then inter per pair into one psum
            psn = ps_n.tile([128, 260], f32, tag="n")
            for h in range(HC):
                nc.tensor.matmul(psn[:, 65 * h:65 * (h + 1)],
                                 AmT[:, 128 * h:128 * (h + 1)],
                                 vs[c][:, 65 * h:65 * (h + 1)],
                                 start=(h == 0),
                                 stop=(c == 0 and h == HC - 1))
            if c > 0:
                for t in range(2):
                    nc.tensor.matmul(psn[:, 130 * t:130 * (t + 1)],
                                     qT[t][:, csl],
                                     Sblk[c % 2][:, 130 * t:130 * (t + 1)],
                                     start=False, stop=(t == 1))
            # state update for chunk c: the psum group must close every chunk
            # (psum cannot be read mid-group), so re-inject the previous
            # state via an identity matmul, then add chunk c's outer products.
            if c < NCH - 1:
                if c > 0:
                    for t in range(2):
                        nc.tensor.matmul(psS[:, 130 * t:130 * (t + 1)],
                                         ident[:],
                                         Sblk[c % 2][:, 130 * t:130 * (t + 1)],
                                         start=(t == 0), stop=False)
                for t in range(2):
                    nc.tensor.matmul(psS[:, 130 * t:130 * (t + 1)],
                                     klv[c // 4][:, c % 4, t, :],
                                     vs[c][:, 130 * t:130 * (t + 1)],
                                     start=(c == 0 and t == 0),
                                     stop=(t == 1))
            # dens -> reciprocal -> scale (split DVE/ACT)
            d4 = work.tile([128, 4], f32, tag="d4")
            nc.vector.tensor_scalar_max(
                d4[:],
                psn.rearrange("p (h w) -> p h w", w=65)[:, :, 64:65].opt(),
                DEPS)
            r4 = work.tile([128, 4], f32, tag="r4")
            nc.vector.reciprocal(r4[:], d4[:])
            att = work.tile([128, 256], bf16, tag="att")
            for h in range(HC):
                dst = att[:, 64 * h:64 * (h + 1)]
                src = psn[:, 65 * h:65 * h + 64]
                sc = r4[:, h:h + 1]
                if h < 2:
                    nc.vector.tensor_scalar_mul(dst, src, sc)
                else:
                    nc.scalar.mul(dst, src, sc)
            # transpose to feature-major (separate psum tiles per half so the
            # copy of one half never races the other's accumulation group)
            psT0 = ps_med.tile([128, 128], bf16, tag="medT", bufs=1)
            nc.tensor.transpose(psT0[:], att[:, 0:128], ident[:])
            nc.scalar.copy(attnT[0][:, csl], psT0[:])
            psT1 = ps_med.tile([128, 128], bf16, tag="medT", bufs=1)
            nc.tensor.transpose(psT1[:], att[:, 128:256], ident[:])
            nc.vector.tensor_copy(attnT[1][:, csl], psT1[:])

        def out_stripe(lc):
            lsl = slice(LCH * lc, LCH * (lc + 1))
            for ot in range(ET):
                ps = ps_big.tile([128, LCH], f32, tag="big")
                for eb in range(2):
                    nc.tensor.matmul(ps[:], wov[:, eb, 128 * ot:128 * (ot + 1)],
                                     attnT[eb][:, lsl],
                                     start=(eb == 0), stop=(eb == 1))
                ob = workb.tile([128, LCH], bf16, tag="ob")
                nc.vector.tensor_copy(ob[:], ps[:])
                nc.gpsimd.dma_start(outT_d[128 * ot:128 * (ot + 1), lsl], ob[:])

        # ---- emission order (the tile scheduler refines per-engine order) ----
        for lt in range(4):
            v_tile(lt)
        qk_stripe(0)
        kl_transposes(0)
        qk_stripe(1)
        for lt in range(4, 8):
            v_tile(lt)
        kl_transposes(1)
        for c in range(0, 4):
            att_chunk(c)
        qk_stripe(2)
        for lt in range(8, 12):
            v_tile(lt)
        kl_transposes(2)
        out_stripe(0)
        for c in range(4, 8):
            att_chunk(c)
        qk_stripe(3)
        for lt in range(12, 16):
            v_tile(lt)
        kl_transposes(3)
        out_stripe(1)
        for c in range(8, 12):
            att_chunk(c)
        out_stripe(2)
        for c in range(12, 16):
            att_chunk(c)
        out_stripe(3)

    nc.compile()
    return nc


def _prep_inputs(x, qkv_w, qkv_b, out_w):
    mask = np.tile(np.triu(np.ones((128, 128), np.float32)), (1, 4))
    ident = np.eye(128, dtype=np.float32).astype(BF)
    in_maps = []
    def etpack(w):
        # (E, C) -> (128, ET*C): et-major packing, [p, et*C + c] = w[128et+p, c]
        e, cc = w.shape
        return w.reshape(e // 128, 128, cc).transpose(1, 0, 2).reshape(128, -1)

    for c in range(NCORES):
        b, hg = c // 4, c % 4
        rows = np.arange(256 * hg, 256 * (hg + 1))
        wqk = np.concatenate([qkv_w[rows], qkv_w[rows + E]], 0).T
        bqk = np.concatenate([qkv_b[rows], qkv_b[rows + E]]).reshape(4, 128).T
        wv = qkv_w[rows + 2 * E].T
        bv = np.tile(qkv_b[rows + 2 * E][None, :], (128, 1))
        wo = out_w[:, rows].T
        in_maps.append({
            "xT": np.ascontiguousarray(x[b].T).astype(BF),
            "wqk": np.ascontiguousarray(etpack(wqk)).astype(BF),
            "bqk": np.ascontiguousarray(bqk).astype(np.float32),
            "wv": np.ascontiguousarray(etpack(wv)).astype(BF),
            "bv": np.ascontiguousarray(bv).astype(BF),
            "wo": np.ascontiguousarray(etpack(wo)).astype(BF),
            "mask": mask, "ident": ident,
        })
    return in_maps


def kernel(x, qkv_w, qkv_b, out_w, out_b):
    from concourse.bass_utils import run_bass_kernel_spmd

    x = np.asarray(x, np.float32)
    qkv_w = np.asarray(qkv_w, np.float32)
    qkv_b = np.asarray(qkv_b, np.float32)
    out_w = np.asarray(out_w, np.float32)
    out_b = np.asarray(out_b, np.float32)

    if "nc" not in _STATE:
        _STATE["nc"] = _build()
    nc = _STATE["nc"]
    in_maps = _prep_inputs(x, qkv_w, qkv_b, out_w)
    res = run_bass_kernel_spmd(nc, in_maps, list(range(NCORES)),
                               trace=PROFILE)
    _STATE["last"] = res
    out = np.zeros((B, L, E), np.float32)
    for c in range(NCORES):
        out[c // 4] += res.results[c]["outT"].T
    out += out_b
    return out


# revision 25
# speedup vs baseline: 1.4035x; 1.1863x over previous
"""Causal linear multi-head attention (decoupled phi) on 8 trn2 NeuronCores.

Sharding: core c handles batch b = c//4 and head group hg = c%4 (4 of 16 heads).
Each core computes qkv projections for its heads, chunked causal linear
attention, and a partial output projection over its 256 feature columns.
Host sums the 4 partials per batch and adds out_b.

v5 layout/schedule notes (all bf16):
  - qkT: one (128, 4, L) tile [q pair0 | q pair1 | k pair0 | k pair1], rows =
    2 heads x 64 feats stacked. qkTo: (64, 4, L) odd-head halves shifted to
    base partition 0 via ONE SBUF-SBUF dma per stripe (base-64 matmul
    operands are not safe on hw).
  - A^T per head via 64-contraction half matmuls; [V|1] rhs carries den.
  - state S lives in PSUM but its accumulation group closes every chunk:
    previous state is re-injected via an identity matmul (PSUM cannot be
    read mid-group), then chunk outer products accumulate. 2 ACT copies
    build the block-diagonal bf16 [S|kcum] used by 2 pair inter matmuls.
  - attention output: PE transpose to feature-major, ONE merged copy per
    chunk into a single (128, 2, L) attnT tile.
  - out-projection interleaved stripe-wise with attention; 2 psum tiles cast
    into one (128, 2, 512) ob tile -> ONE dma per 2 output row-blocks.
  - dma queues: scalar = inputs; sync = x0/wv + kl transposes (batched
    per-stripe 3D transposes); gpsimd = odd-half shifts + output dmas.
    Few, large dmas keep the shared dma-semaphore pool uncontended.
"""

import numpy as np
import ml_dtypes

BF = ml_dtypes.bfloat16

B, L, E, H, D = 2, 2048, 1024, 16, 64
HC = 4            # heads per core
NCORES = 8
CH = 128          # chunk length
NCH = L // CH     # 16 chunks
LCH = 512         # l stripe for projections
NLC = L // LCH    # 4
ET = E // 128     # 8 e-tiles
DEPS = 1e-6

PROFILE = False
_STATE = {}


def _build():
    from contextlib import ExitStack
    from concourse import bacc, tile, mybir

    f32 = mybir.dt.float32
    bf16 = mybir.dt.bfloat16

    nc = bacc.Bacc("TRN2", target_bir_lowering=False, debug=False,
                   num_devices=NCORES)

    xT_d = nc.dram_tensor("xT", [E, L], bf16, kind="ExternalInput").ap()
    wqk_d = nc.dram_tensor("wqk", [128, ET * 512], bf16, kind="ExternalInput").ap()
    bqk_d = nc.dram_tensor("bqk", [128, 4], f32, kind="ExternalInput").ap()
    wv_d = nc.dram_tensor("wv", [128, ET * 256], bf16, kind="ExternalInput").ap()
    bv_d = nc.dram_tensor("bv", [128, 256], bf16, kind="ExternalInput").ap()
    wo_d = nc.dram_tensor("wo", [128, 2 * E], bf16, kind="ExternalInput").ap()
    mask_d = nc.dram_tensor("mask", [128, 512], f32, kind="ExternalInput").ap()
    ident_d = nc.dram_tensor("ident", [128, 128], bf16, kind="ExternalInput").ap()
    outT_d = nc.dram_tensor("outT", [E, L], bf16, kind="ExternalOutput").ap()

    with tile.TileContext(nc) as tc, ExitStack() as ctx:
        persist = ctx.enter_context(tc.tile_pool(name="persist", bufs=1))
        ps_big = ctx.enter_context(tc.tile_pool(name="psbig", bufs=2, space="PSUM"))
        ps_med = ctx.enter_context(tc.tile_pool(name="psmed", bufs=2, space="PSUM"))
        ps_n = ctx.enter_context(tc.tile_pool(name="psn", bufs=2, space="PSUM"))
        ps_st = ctx.enter_context(tc.tile_pool(name="psst", bufs=1, space="PSUM"))
        work = ctx.enter_context(tc.tile_pool(name="work", bufs=3))
        workb = ctx.enter_context(tc.tile_pool(name="workb", bufs=8))

        def pt(shape, dt, tag):
            return persist.tile(shape, dt, tag=tag, name=tag)

        # ---- persistent tiles ----
        xTt = pt([128, ET * L], bf16, "xTt")
        xv = xTt.rearrange("p (a l) -> p a l", l=L)
        wqkt = pt([128, ET * 512], bf16, "wqkt")
        wqkv = wqkt.rearrange("p (a c) -> p a c", c=512)
        wvt = pt([128, ET * 256], bf16, "wvt")
        wvv = wvt.rearrange("p (a c) -> p a c", c=256)
        wot = pt([128, 2 * E], bf16, "wot")
        wov = wot.rearrange("p (a c) -> p a c", c=E)
        bqk = pt([128, 4], f32, "bqk")
        bv = pt([128, 256], bf16, "bv")
        mask = pt([128, 512], f32, "mask")
        ident = pt([128, 128], bf16, "ident")

        # [q pair0 | q pair1 | k pair0 | k pair1]
        qkT = pt([128, 4 * L], bf16, "qkT")
        qkv_ = qkT.rearrange("p (a l) -> p a l", l=L)
        qkTo = pt([64, 4 * L], bf16, "qkTo")
        qkvo = qkTo.rearrange("p (a l) -> p a l", l=L)
        # l-major k per stripe: klv[lc][p, j, t, w] = phi(k)[512lc+128j+p, 128t+w]
        klS = [pt([128, 1024], bf16, f"klS{i}") for i in range(NLC)]
        klv = [klS[i].rearrange("p (j t w) -> p j t w", t=2, w=128)
               for i in range(NLC)]
        vs = [pt([128, 260], bf16, f"vs{i}") for i in range(NCH)]
        attnT = pt([128, 2 * L], bf16, "attnT")
        attv = attnT.rearrange("p (a l) -> p a l", l=L)
        Sblk = [pt([128, 260], bf16, f"Sblk{i}") for i in range(2)]

        # ---- input DMAs ----
        xsrc = xT_d.rearrange("(a p) l -> p a l", p=128)
        nc.sync.dma_start(xv[:, :, 0:LCH], xsrc[:, :, 0:LCH])
        nc.scalar.dma_start(wqkt[:], wqk_d[:])
        nc.scalar.dma_start(bqk[:], bqk_d[:])
        nc.scalar.dma_start(bv[:], bv_d[:])
        nc.sync.dma_start(wvt[:], wv_d[:])
        nc.scalar.dma_start(mask[:], mask_d[:])
        nc.scalar.dma_start(ident[:], ident_d[:])
        nc.scalar.dma_start(xv[:, :, LCH:2 * LCH], xsrc[:, :, LCH:2 * LCH])
        nc.scalar.dma_start(xv[:, :, 2 * LCH:3 * LCH], xsrc[:, :, 2 * LCH:3 * LCH])
        nc.scalar.dma_start(xv[:, :, 3 * LCH:4 * LCH], xsrc[:, :, 3 * LCH:4 * LCH])
        nc.scalar.dma_start(wot[:], wo_d[:])

        # zero-init: Sblk off-diagonal stays zero forever; vs ones columns
        for i in range(2):
            nc.gpsimd.memset(Sblk[i][:], 0.0)
        for lt in range(NCH):
            v3 = vs[lt].rearrange("p (h w) -> p h w", w=65)
            nc.gpsimd.memset(v3[:, :, 64:65], 1.0)

        # state psum: pair t diag blocks [S|kcum] at cols 130t (+65 for odd)
        psS = ps_st.tile([128, 260], f32, tag="S")

        # ---- emission helpers ----
        def v_tile(lt):
            ps = ps_med.tile([128, 256], f32, tag="med")
            for et in range(ET):
                nc.tensor.matmul(ps[:], xv[:, et, 128 * lt:128 * (lt + 1)],
                                 wvv[:, et, :], start=(et == 0), stop=(et == ET - 1))
            v3 = vs[lt].rearrange("p (h w) -> p h w", w=65)
            nc.vector.tensor_add(v3[:, :, 0:64],
                                 ps.rearrange("p (h w) -> p h w", w=64),
                                 bv.rearrange("p (h w) -> p h w", w=64))

        def qk_stripe(lc):
            lsl = slice(LCH * lc, LCH * (lc + 1))
            for ct in range(4):
                ps = ps_big.tile([128, LCH], f32, tag="big")
                for et in range(ET):
                    nc.tensor.matmul(
                        ps[:], wqkv[:, et, 128 * ct:128 * (ct + 1)],
                        xv[:, et, lsl], start=(et == 0), stop=(et == ET - 1))
                nc.scalar.activation(qkv_[:, ct, lsl], ps[:],
                                     mybir.ActivationFunctionType.Relu,
                                     bias=bqk[:, ct:ct + 1])
            # odd-head halves to base partition 0, all four ct in one dma
            nc.gpsimd.dma_start(qkvo[:, :, lsl], qkv_[64:128, :, lsl])

        def kl_transposes(lc):
            if lc == 0:
                # per-chunk for latency: chunk 0's state matmul is needed early
                for j in range(4):
                    for t in range(2):
                        nc.sync.dma_start_transpose(
                            klv[0][:, j, t, :],
                            qkv_[:, 2 + t, 128 * j:128 * (j + 1)])
            else:
                for t in range(2):
                    nc.sync.dma_start_transpose(
                        klv[lc][:, :, t, :],
                        qkv_[:, 2 + t, LCH * lc:LCH * (lc + 1)])

        def att_chunk(c):
            csl = slice(128 * c, 128 * (c + 1))
            # A^T per head: (128 lk, 128 lq), 64-feat contraction halves
            psA = ps_big.tile([128, 512], f32, tag="big")
            for h in range(HC):
                t, s = h // 2, h % 2
                if s == 0:
                    lh = qkv_[0:64, 2 + t, csl]
                    rh = qkv_[0:64, t, csl]
                else:
                    lh = qkvo[:, 2 + t, csl]
                    rh = qkvo[:, t, csl]
                nc.tensor.matmul(psA[:, 128 * h:128 * (h + 1)], lh, rh,
                                 start=(h == 0), stop=(h == HC - 1))
            AmT = work.tile([128, 512], bf16, tag="AmT")
            nc.vector.tensor_mul(AmT[:], psA[:], mask[:])
            # block-diag bf16 state [S|kcum] for this chunk's inter matmuls
            if c > 0:
                Sb = Sblk[c % 2]
                for s in range(2):
                    rsl = slice(64 * s, 64 * (s + 1))
                    v3d = lambda ap: ap[rsl, :].rearrange(
                        "p (t w) -> p t w", w=130)[:, :, 65 * s:65 * (s + 1)]
                    nc.scalar.copy(v3d(Sb), v3d(psS))
            # num/den: intra per head then inter per pair into one psum
            psn = ps_n.tile([128, 260], f32, tag="n")
            for h in range(HC):
                nc.tensor.matmul(psn[:, 65 * h:65 * (h + 1)],
                                 AmT[:, 128 * h:128 * (h + 1)],
                                 vs[c][:, 65 * h:65 * (h + 1)],
                                 start=(h == 0),
                                 stop=(c == 0 and h == HC - 1))
            if c > 0:
                for t in range(2):
                    nc.tensor.matmul(psn[:, 130 * t:130 * (t + 1)],
                                     qkv_[:, t, csl],
                                     Sblk[c % 2][:, 130 * t:130 * (t + 1)],
                                     start=False, stop=(t == 1))
            # state update for chunk c: group closes every chunk (psum cannot
            # be read mid-group): re-inject previous state via identity
            # matmul, then add chunk c's outer products.
            if c < NCH - 1:
                if c > 0:
                    for t in range(2):
                        nc.tensor.matmul(psS[:, 130 * t:130 * (t + 1)],
                                         ident[:],
                                         Sblk[c % 2][:, 130 * t:130 * (t + 1)],
                                         start=(t == 0), stop=False)
                for t in range(2):
                    nc.tensor.matmul(psS[:, 130 * t:130 * (t + 1)],
                                     klv[c // 4][:, c % 4, t, :],
                                     vs[c][:, 130 * t:130 * (t + 1)],
                                     start=(c == 0 and t == 0),
                                     stop=(t == 1))
            # dens -> reciprocal -> scale (split DVE/ACT)
            d4 = work.tile([128, 4], f32, tag="d4")
            nc.vector.tensor_scalar_max(
                d4[:],
                psn.rearrange("p (h w) -> p h w", w=65)[:, :, 64:65].opt(),
                DEPS)
            r4 = work.tile([128, 4], f32, tag="r4")
            nc.vector.reciprocal(r4[:], d4[:])
            att = work.tile([128, 256], bf16, tag="att")
            for h in range(HC):
                dst = att[:, 64 * h:64 * (h + 1)]
                src = psn[:, 65 * h:65 * h + 64]
                sc = r4[:, h:h + 1]
                if h < 2:
                    nc.vector.tensor_scalar_mul(dst, src, sc)
                else:
                    nc.scalar.mul(dst, src, sc)
            # transpose to feature-major; ONE merged copy into attnT
            psT = ps_med.tile([128, 256], bf16, tag="medT", bufs=1)
            nc.tensor.matmul(psT[:, 0:128], att[:, 0:128], ident[:],
                             is_transpose=True, start=True, stop=False)
            nc.tensor.matmul(psT[:, 128:256], att[:, 128:256], ident[:],
                             is_transpose=True, start=False, stop=True)
            nc.vector.tensor_copy(attv[:, :, csl],
                                  psT.rearrange("p (a w) -> p a w", w=128))

        def out_stripe(lc):
            lsl = slice(LCH * lc, LCH * (lc + 1))
            for og in range(ET // 2):
                ob = workb.tile([128, 2 * LCH], bf16, tag="ob")
                for j in range(2):
                    ot = 2 * og + j
                    ps = ps_big.tile([128, LCH], f32, tag="big")
                    for eb in range(2):
                        nc.tensor.matmul(
                            ps[:], wov[:, eb, 128 * ot:128 * (ot + 1)],
                            attv[:, eb, lsl], start=(eb == 0), stop=(eb == 1))
                    if j == 0:
                        nc.vector.tensor_copy(ob[:, 0:LCH], ps[:])
                    else:
                        nc.scalar.copy(ob[:, LCH:2 * LCH], ps[:])
                dst = outT_d[256 * og:256 * (og + 1), lsl]
                nc.gpsimd.dma_start(
                    dst.rearrange("(a p) l -> p a l", p=128),
                    ob.rearrange("p (a l) -> p a l", l=LCH))

        # ---- emission order (the tile scheduler refines per-engine order) ----
        for lt in range(4):
            v_tile(lt)
        qk_stripe(0)
        kl_transposes(0)
        qk_stripe(1)
        for lt in range(4, 8):
            v_tile(lt)
        kl_transposes(1)
        for c in range(0, 4):
            att_chunk(c)
        qk_stripe(2)
        for lt in range(8, 12):
            v_tile(lt)
        kl_transposes(2)
        out_stripe(0)
        for c in range(4, 8):
            att_chunk(c)
        qk_stripe(3)
        for lt in range(12, 16):
            v_tile(lt)
        kl_transposes(3)
        out_stripe(1)
        for c in range(8, 12):
            att_chunk(c)
        out_stripe(2)
        for c in range(12, 16):
            att_chunk(c)
        out_stripe(3)

    nc.compile()
    return nc


def _prep_inputs(x, qkv_w, qkv_b, out_w):
    mask = np.tile(np.triu(np.ones((128, 128), np.float32)), (1, 4))
    ident = np.eye(128, dtype=np.float32).astype(BF)

    def etpack(w):
        # (E, C) -> (128, ET*C): et-major packing, [p, et*C + c] = w[128et+p, c]
        e, cc = w.shape
        return w.reshape(e // 128, 128, cc).transpose(1, 0, 2).reshape(128, -1)

    in_maps = []
    for c in range(NCORES):
        b, hg = c // 4, c % 4
        rows = np.arange(256 * hg, 256 * (hg + 1))
        wqk = np.concatenate([qkv_w[rows], qkv_w[rows + E]], 0).T
        bqk = np.concatenate([qkv_b[rows], qkv_b[rows + E]]).reshape(4, 128).T
        wv = qkv_w[rows + 2 * E].T
        bv = np.tile(qkv_b[rows + 2 * E][None, :], (128, 1))
        wo = out_w[:, rows].T
        in_maps.append({
            "xT": np.ascontiguousarray(x[b].T).astype(BF),
            "wqk": np.ascontiguousarray(etpack(wqk)).astype(BF),
            "bqk": np.ascontiguousarray(bqk).astype(np.float32),
            "wv": np.ascontiguousarray(etpack(wv)).astype(BF),
            "bv": np.ascontiguousarray(bv).astype(BF),
            "wo": np.ascontiguousarray(etpack(wo)).astype(BF),
            "mask": mask, "ident": ident,
        })
    return in_maps


def kernel(x, qkv_w, qkv_b, out_w, out_b):
    from concourse.bass_utils import run_bass_kernel_spmd

    x = np.asarray(x, np.float32)
    qkv_w = np.asarray(qkv_w, np.float32)
    qkv_b = np.asarray(qkv_b, np.float32)
    out_w = np.asarray(out_w, np.float32)
    out_b = np.asarray(out_b, np.float32)

    if "nc" not in _STATE:
        _STATE["nc"] = _build()
    nc = _STATE["nc"]
    in_maps = _prep_inputs(x, qkv_w, qkv_b, out_w)
    res = run_bass_kernel_spmd(nc, in_maps, list(range(NCORES)),
                               trace=PROFILE)
    _STATE["last"] = res
    out = np.zeros((B, L, E), np.float32)
    for c in range(NCORES):
        out[c // 4] += res.results[c]["outT"].T
    out += out_b
    return out
